# revision 52
# baseline (speedup 1.0000x reference)
"""Trainium2 Bass kernel for DeepGraphConvolution message passing.

Computes, for full inputs:
    hidden  = x2 @ W.T + b
    support = segment_sum(edge_vals[:,None] * hidden[cols], rows)
    y1      = relu(support) + x1
    y2      = x2
    returns (y2, y1)

Strategy (8 NeuronCores, SPMD, no collectives needed):
    support = (A @ x2) @ W.T + deg x b       where A[i,j] = sum of edge_vals
    over edges (i<-j), deg[i] = sum of edge_vals into i.

    Nodes (rows of x1/x2/support) are sharded across the 8 cores; edges are
    partitioned by destination row (standard 1D graph partitioning).  Each
    core holds a full copy of x2 in DRAM (bf16) and gathers the source rows
    its edges need with GPSIMD dma_gather (one descriptor per edge slot,
    128 slots per chunk).  A per-chunk selection matrix
    S[k, n] = v_k * (ldest_k == n) is built ON-DEVICE by the vector engine
    (iota table + two broadcast tensor_tensor passes) and the tensor engine
    accumulates  accT[f, n] += G[k, f]^T S[k, n]  over each 128-destination
    block in PSUM (f32).  The small 256x256 weight is applied per block in
    f32, the deg x b rank-1 term is added with a K=1 matmul, then relu + x1.

    dma_gather indices are int16, so the source table is split in two
    halves (lo: nodes [0, 32704), hi: the rest) and each block's slot list
    is ordered [lo-slots | pad | hi-slots | pad] with chunk counts fixed
    across cores (max over cores, padded with idx=0 / v=0).

    GPSIMD SWDGE descriptor generation (~8 ns/descriptor single-queue) is
    the fundamental bottleneck; the gather calls are striped round-robin
    over the 4 SWDGE queues (queue_num selects which GPSIMD Q7 core pair
    generates descriptors, and pairs run concurrently: ~2.4x measured).

    Tuning beyond the original baseline (408-435us -> ~340us):
      - 4 persistent gather buffers + deeper sel/out tile pools so the
        gather/S-build/matmul/W stages of ~4 blocks pipeline.
      - PSUM->SBUF accumulator copies moved from DVE to the mostly-idle
        ACT engine (DVE was the densest engine: is_equal + mult S-build).
      - Slots sorted by source index within each (block, half) group.
      - x1/y1 staged in bf16 (halves the non-gather DMA traffic).
      - iota/local-dest tables in int8.
      - Per-core block-position balancing (sort blocks by edge count).
    Known dead ends (measured): single_packet=False or calls >1008
    descriptors (64-desc/engine packet limit -> device crash at >1008;
    slower when split smaller or merged bigger); trailing idx=-1 pad
    trimming (ucode supports it but the runtime ring bookkeeping
    desyncs -> crash); fp8 gather/matmul (rel err 3.2e-2 > 2e-2 gate);
    int8 S-build inputs gave no DVE speedup (broadcast APs stay 1x).
"""

import os
import numpy as np

P = 128
D = 256
M = 8  # NeuronCores

_NC_CACHE: dict = {}


def _build_nc(CA, CB, C_total, S16, lo, nhi, npad, gdt_name):
    """Build the Bass program. CA/CB: per-block chunk counts (lo/hi half)."""
    import concourse.bacc as bacc
    import concourse.tile as tile
    from concourse import mybir
    from concourse.alu_op_type import AluOpType

    NACT = int(os.environ.get("GNN_NACT", "0"))
    NGBUF = int(os.environ.get("GNN_NGBUF", "6"))
    NBLK = len(CA)
    gdt = getattr(mybir.dt, gdt_name)
    f32 = mybir.dt.float32

    nc = bacc.Bacc(
        "TRN2",
        target_bir_lowering=False,
        debug=False,
        num_devices=M,
        num_swdge_queues=4,
    )

    x2lo = nc.dram_tensor("x2lo", [lo, D], gdt, kind="ExternalInput").ap()
    x2hi = nc.dram_tensor("x2hi", [nhi, D], gdt, kind="ExternalInput").ap()
    idx = nc.dram_tensor("idx", [P, S16], mybir.dt.int16, kind="ExternalInput").ap()
    # per-slot local-dest + edge-value tables (S is built on-device)
    i8 = mybir.dt.int8
    ldw = nc.dram_tensor("ldw", [P, C_total], i8, kind="ExternalInput").ap()
    vvw = nc.dram_tensor("vvw", [P, C_total], gdt, kind="ExternalInput").ap()
    vvf = nc.dram_tensor("vvf", [P, C_total], f32, kind="ExternalInput").ap()
    iot = nc.dram_tensor("iot", [P, P], i8, kind="ExternalInput").ap()
    deg = nc.dram_tensor("deg", [1, npad], f32, kind="ExternalInput").ap()
    bf16 = mybir.dt.bfloat16
    x1s = nc.dram_tensor("x1s", [npad, D], bf16, kind="ExternalInput").ap()
    wt = nc.dram_tensor("wt", [P, 2 * D], f32, kind="ExternalInput").ap()
    bb = nc.dram_tensor("bb", [1, D], f32, kind="ExternalInput").ap()
    y1s = nc.dram_tensor("y1s", [npad, D], bf16, kind="ExternalOutput").ap()

    # chunk base per block
    CAB = [a + b_ for a, b_ in zip(CA, CB)]
    chb = np.concatenate([[0], np.cumsum(CAB)]).astype(int)
    CMAX = int(max(CAB))

    with tile.TileContext(nc) as tc:
        from contextlib import ExitStack

        with ExitStack() as ctx:
            cpool = ctx.enter_context(tc.tile_pool(name="const", bufs=1))
            spool = ctx.enter_context(tc.tile_pool(name="sel", bufs=12))
            pspool = ctx.enter_context(tc.tile_pool(name="ps", bufs=2, space="PSUM"))
            p2pool = ctx.enter_context(
                tc.tile_pool(name="ps2", bufs=3, space="PSUM")
            )
            apool = ctx.enter_context(tc.tile_pool(name="accs", bufs=4))
            opool = ctx.enter_context(tc.tile_pool(name="outs", bufs=8))

            # --- constants ---
            # idx table is split so the first blocks' gathers don't wait
            # for the full 1.7MB load (cuts ~8us of pipeline ramp); loads
            # are ordered by when the pipeline needs them.
            chb_l = np.concatenate([[0], np.cumsum(CAB)]).astype(int)
            split_blk = min(4, NBLK)
            split16 = int(chb_l[split_blk]) * 8
            idx_a = cpool.tile([P, split16], mybir.dt.int16, name="idx_a")
            nc.sync.dma_start(idx_a[:], idx[:, 0:split16])
            ldw_sb = cpool.tile([P, C_total], i8)
            nc.sync.dma_start(ldw_sb[:], ldw[:, :])
            iot_sb = cpool.tile([P, P], i8)
            nc.sync.dma_start(iot_sb[:], iot[:, :])
            vvw_sb = cpool.tile([P, C_total], gdt)
            nc.sync.dma_start(vvw_sb[:], vvw[:, :])
            idx_b = cpool.tile([P, S16 - split16], mybir.dt.int16, name="idx_b")
            nc.sync.dma_start(idx_b[:], idx[:, split16:S16])
            wt_sb = cpool.tile([P, 2 * D], f32)
            nc.sync.dma_start(wt_sb[:], wt[:, :])
            b_sb = cpool.tile([1, D], f32)
            nc.sync.dma_start(b_sb[:], bb[:, :])
            deg_sb = cpool.tile([1, npad], f32)
            nc.sync.dma_start(deg_sb[:], deg[:, :])
            vvf_sb = cpool.tile([P, C_total], f32)
            nc.sync.dma_start(vvf_sb[:], vvf[:, :])

            # persistent multi-buffered gather tiles
            gbufs = [
                cpool.tile([P, CMAX, D], gdt, tag=f"g_{i}", name=f"g_{i}")
                for i in range(NGBUF)
            ]

            call_no = 0
            for b in range(NBLK):
                nchunks = CAB[b]
                g = gbufs[b % NGBUF][:, 0:nchunks, :]
                # gather source rows for this block's slots; split calls to
                # respect the SWDGE packet limit (single_packet=True concats
                # a call's per-engine descriptors into one packet, max 64
                # descs -> num_idxs <= 1008), striped over the 4 SWDGE queues
                GMAX = int(os.environ.get("GNN_GMAX", "7"))
                sp = os.environ.get("GNN_SP", "1") == "1"
                for src_ap, cnt, coff in (
                    (x2lo, CA[b], 0),
                    (x2hi, CB[b], CA[b]),
                ):
                    for o in range(0, cnt, GMAX):
                        n = min(GMAX, cnt - o)
                        ni = n * P
                        off16 = (chb[b] + coff + o) * P // 16
                        if b < split_blk:
                            idxs_ap = idx_a[:, off16 : off16 + n * 8]
                        else:
                            idxs_ap = idx_b[
                                :, off16 - split16 : off16 - split16 + n * 8
                            ]
                        nc.gpsimd.dma_gather(
                            out_ap=g[:, coff + o : coff + o + n, :],
                            in_ap=src_ap,
                            idxs_ap=idxs_ap,
                            num_idxs=ni,
                            num_idxs_reg=ni,
                            elem_size=D,
                            queue_num=call_no % 4,
                            single_packet=sp,
                        )
                        call_no += 1

                # build S on-device: S[k, c, n] = (iota_n == ld[k,c]) * v[k,c]
                s_blk = spool.tile([P, nchunks, P], gdt, tag="s")
                iota_b = iot_sb[:].unsqueeze(1).broadcast_to([P, nchunks, P])
                ld_b = (
                    ldw_sb[:, chb[b] : chb[b] + nchunks]
                    .unsqueeze(2)
                    .broadcast_to([P, nchunks, P])
                )
                vv_b = (
                    vvw_sb[:, chb[b] : chb[b] + nchunks]
                    .unsqueeze(2)
                    .broadcast_to([P, nchunks, P])
                )
                nc.vector.tensor_tensor(
                    out=s_blk[:, :, :], in0=iota_b, in1=ld_b,
                    op=AluOpType.is_equal,
                )
                nc.vector.tensor_tensor(
                    out=s_blk[:, :, :], in0=s_blk[:, :, :], in1=vv_b,
                    op=AluOpType.mult,
                )

                pt0 = pspool.tile([P, P], f32, tag="pt0")
                pt1 = pspool.tile([P, P], f32, tag="pt1")
                for ci in range(nchunks):
                    s = s_blk[:, ci, :]
                    st = ci == 0
                    sp = ci == nchunks - 1
                    nc.tensor.matmul(
                        out=pt0[:], lhsT=g[:, ci, 0:P], rhs=s, start=st, stop=sp
                    )
                    nc.tensor.matmul(
                        out=pt1[:], lhsT=g[:, ci, P:D], rhs=s, start=st, stop=sp
                    )

                a0 = apool.tile([P, P], f32, tag="a0")
                a1 = apool.tile([P, P], f32, tag="a1")
                # PSUM -> SBUF on the (mostly idle) ACT engine, not DVE
                nc.scalar.activation(a0[:], pt0[:], mybir.ActivationFunctionType.Copy)
                nc.scalar.activation(a1[:], pt1[:], mybir.ActivationFunctionType.Copy)

                p2 = p2pool.tile([P, D], f32, tag="p2")
                nc.tensor.matmul(
                    out=p2[:], lhsT=a0[:], rhs=wt_sb[:, 0:D], start=True, stop=False
                )
                nc.tensor.matmul(
                    out=p2[:], lhsT=a1[:], rhs=wt_sb[:, D : 2 * D], start=False,
                    stop=False,
                )
                nc.tensor.matmul(
                    out=p2[:],
                    lhsT=deg_sb[:, b * P : (b + 1) * P],
                    rhs=b_sb[:],
                    start=False,
                    stop=True,
                )

                r = opool.tile([P, D], bf16, tag="r")
                nc.scalar.activation(r[:], p2[:], mybir.ActivationFunctionType.Relu)
                x1t = opool.tile([P, D], bf16, tag="x1t")
                nc.sync.dma_start(x1t[:], x1s[b * P : (b + 1) * P, :])
                y = opool.tile([P, D], bf16, tag="y")
                nc.vector.tensor_add(y[:], r[:], x1t[:])
                nc.sync.dma_start(y1s[b * P : (b + 1) * P, :], y[:])

    nc.compile()
    return nc


def _plan(rows, cols, edge_vals, x1, x2, W, b, n_nodes, lo, gdt_np):
    """Host-side sharding: per-core padded slot structure + input maps."""
    E = rows.shape[0]
    nper = n_nodes // M
    nblk = -(-nper // P)
    npad = nblk * P
    nhi = n_nodes - lo

    rows = rows.astype(np.int64)
    cols = cols.astype(np.int64)
    v = edge_vals.astype(np.float32)

    core = rows // nper
    ldest = rows - core * nper
    blk = ldest // P
    half = (cols >= lo).astype(np.int64)
    gid0 = (core * nblk + blk) * 2 + half
    ngroups = M * nblk * 2
    cnt0 = np.bincount(gid0, minlength=ngroups).reshape(M, nblk, 2)

    # per-core block ordering: sort blocks by total edge count (descending)
    # so position i has similar counts across cores -> smaller max -> less
    # chunk padding.  blkperm[r, i] = original block processed at position i.
    tot0 = cnt0[:, :, 0] + cnt0[:, :, 1]
    blkperm = np.argsort(-tot0, axis=1, kind="stable")  # [M, nblk]
    posof = np.empty_like(blkperm)
    for r in range(M):
        posof[r, blkperm[r]] = np.arange(nblk)
    pos_blk = posof[core, blk]  # position of each edge's block in its core
    gid = (core * nblk + pos_blk) * 2 + half
    cnt = np.bincount(gid, minlength=ngroups).reshape(M, nblk, 2)

    # fixed per-position chunk counts = max over cores, ceil to 128
    CA = np.maximum(-(-cnt[:, :, 0].max(axis=0) // P), 0).astype(int)
    CB = np.maximum(-(-cnt[:, :, 1].max(axis=0) // P), 0).astype(int)
    for bi in range(nblk):
        if CA[bi] + CB[bi] == 0:
            CA[bi] = 1  # keep >=1 chunk per block so PSUM is initialized
    CAB = CA + CB
    C_total = int(CAB.sum())
    S = C_total * P  # slots per core
    S16 = S // 16

    # slot base per (block, half), identical across cores
    chb = np.concatenate([[0], np.cumsum(CAB)]).astype(np.int64)
    baseA = chb[:-1] * P
    baseB = baseA + CA * P
    base_bh = np.stack([baseA, baseB], axis=1)  # [nblk, 2]

    # rank of each edge within its (core, blk, half) group; secondary sort
    # by col for better HBM locality in the gather stream
    order = np.lexsort((cols, gid))
    gsort = gid[order]
    flat_cnt = cnt.reshape(-1)
    starts = np.concatenate([[0], np.cumsum(flat_cnt)])[:-1]
    rank_sorted = np.arange(E, dtype=np.int64) - starts[gsort]
    rank = np.empty(E, dtype=np.int64)
    rank[order] = rank_sorted

    pos = core * S + base_bh[pos_blk, half] + rank  # [E] in [0, M*S)

    # Pad slots use idx=-1: the gather ucode drops trailing negative
    # indices (fewer descriptors); their S entries are 0 so stale SBUF
    # data contributes nothing.  Positions 0-2 (first use of each of the
    # 3 gather buffers, the largest blocks) keep idx=0 pads so every slot
    # of the buffer range is initialized with finite data once.
    if os.environ.get("GNN_NEGPAD", "0") == "1":
        idx_flat = np.full(M * S, -1, dtype=np.int16)
        for i in range(min(3, nblk)):
            for r in range(M):
                idx_flat[r * S + chb[i] * P : r * S + chb[i + 1] * P] = 0
    else:
        idx_flat = np.zeros(M * S, dtype=np.int16)
    ld_flat = np.zeros(M * S, dtype=np.float32)
    v_flat = np.zeros(M * S, dtype=np.float32)
    idx_flat[pos] = np.where(half == 1, cols - lo, cols).astype(np.int16)
    ld_flat[pos] = (ldest % P).astype(np.float32)
    v_flat[pos] = v

    # weight: wt[p, t*256+n] = W[n, t*128+p]
    wt_host = np.ascontiguousarray(
        W.astype(np.float32).T.reshape(2, P, D).transpose(1, 0, 2).reshape(P, 2 * D)
    )
    b_host = np.ascontiguousarray(b.astype(np.float32).reshape(1, D))
    x2lo_host = np.ascontiguousarray(x2[:lo].astype(gdt_np))
    x2hi_host = np.ascontiguousarray(x2[lo:].astype(gdt_np))
    iot_host = np.ascontiguousarray(
        np.tile(np.arange(P, dtype=np.int8), (P, 1))
    )

    in_maps = []
    for r in range(M):
        sl = slice(r * S, (r + 1) * S)
        idx_w = idx_flat[sl].reshape(S16, 16).T  # [16, S16]
        idx_w = np.ascontiguousarray(np.tile(idx_w, (8, 1)))  # [128, S16]
        # per-slot tables wrapped [k, c]: slot s = (c, k) with k = s % 128
        ld_w = np.ascontiguousarray(
            ld_flat[sl].reshape(C_total, P).T.astype(np.int8)
        )
        vv_w = np.ascontiguousarray(
            v_flat[sl].reshape(C_total, P).T.astype(gdt_np)
        )
        vv_f = np.ascontiguousarray(v_flat[sl].reshape(C_total, P).T)
        msk = core == r
        deg_orig = np.bincount(ldest[msk], weights=v[msk], minlength=npad)[
            :npad
        ].astype(np.float32)
        import ml_dtypes

        x1_orig = np.zeros((npad, D), dtype=ml_dtypes.bfloat16)
        x1_orig[:nper] = x1[r * nper : (r + 1) * nper].astype(ml_dtypes.bfloat16)
        # permute per-core rows: position i holds original block blkperm[r, i]
        rowmap = (
            blkperm[r][:, None] * P + np.arange(P)[None, :]
        ).reshape(-1)  # [npad]
        deg_h = deg_orig[rowmap].reshape(1, npad).copy()
        x1_h = np.ascontiguousarray(x1_orig[rowmap])
        in_maps.append(
            {
                "x2lo": x2lo_host,
                "x2hi": x2hi_host,
                "idx": idx_w,
                "ldw": ld_w,
                "vvw": vv_w,
                "vvf": vv_f,
                "iot": iot_host,
                "deg": deg_h,
                "x1s": x1_h,
                "wt": wt_host,
                "bb": b_host,
            }
        )

    meta = dict(
        CA=tuple(int(x) for x in CA),
        CB=tuple(int(x) for x in CB),
        C_total=C_total,
        S16=S16,
        lo=lo,
        nhi=nhi,
        npad=npad,
        nper=nper,
        blkperm=blkperm,
    )
    return in_maps, meta


def _unshard(results, meta):
    """Concatenate per-core y1s outputs, undoing the block permutation."""
    nper = meta["nper"]
    npad = meta["npad"]
    blkperm = meta["blkperm"]
    parts = []
    for r in range(M):
        y_perm = results[r]["y1s"]  # [npad, D], position-ordered
        y_orig = np.empty((npad, y_perm.shape[1]), dtype=y_perm.dtype)
        rowmap = (blkperm[r][:, None] * P + np.arange(P)[None, :]).reshape(-1)
        y_orig[rowmap] = y_perm
        parts.append(y_orig[:nper])
    return np.concatenate(parts, axis=0)


def _get_nc(meta, gdt_name):
    key = (meta["CA"], meta["CB"], meta["S16"], meta["lo"], meta["npad"], gdt_name)
    if key not in _NC_CACHE:
        _NC_CACHE[key] = _build_nc(
            list(meta["CA"]),
            list(meta["CB"]),
            meta["C_total"],
            meta["S16"],
            meta["lo"],
            meta["nhi"],
            meta["npad"],
            gdt_name,
        )
    return _NC_CACHE[key]


def _gdt(gdt_name):
    if gdt_name == "bfloat16":
        import ml_dtypes

        return ml_dtypes.bfloat16
    return np.float32


def kernel(x1, x2, rows, cols, edge_vals, W, b):
    from concourse.bass_utils import run_bass_kernel_spmd

    x1 = np.asarray(x1)
    x2 = np.asarray(x2)
    rows = np.asarray(rows)
    cols = np.asarray(cols)
    edge_vals = np.asarray(edge_vals)
    W = np.asarray(W)
    b = np.asarray(b)

    n_nodes = x1.shape[0]
    gdt_name = os.environ.get("GNN_GDT", "bfloat16")
    gdt_np = _gdt(gdt_name)

    # Both halves must fit int16 gather indices. Asymmetric split: the hi
    # half (~35% of edges, ~6 chunks/block) then fits one <=896-descriptor
    # dma_gather call per block, minimizing the ~1us/call fixed cost.
    lo = (n_nodes + 1) // 2 if n_nodes <= 32704 else 32704
    assert lo <= 32767 and n_nodes - lo <= 32767
    in_maps, meta = _plan(
        rows, cols, edge_vals, x1, x2, W, b, n_nodes, lo, gdt_np
    )
    nc = _get_nc(meta, gdt_name)

    res = run_bass_kernel_spmd(nc, in_maps, core_ids=list(range(M)))

    y1 = _unshard(res.results, meta)
    y2 = x2.astype(np.float32)
    return (y2, y1.astype(np.float32))

